# revision 38
# baseline (speedup 1.0000x reference)
"""Trainium2 Bass kernel for nn_MixUniformAffineQuantizer.

kernel(x, upbound_factor, lowbound_factor) -> [4096, 11008] f32.

Strategy: rows sharded 512/core across 8 NeuronCores (row-parallel, no
communication). Per core, per [128-row x 5504-col] chunk:
  - DVE 3D-view tensor_reduce: per-group min/max (+ sum/|sum| for the
    ternary group 0)
  - per-group scale / zero-point math on [128, 43] stat tiles, replicating
    the jax/XLA eager reference (XLA lowers f32 divide as
    multiply-by-reciprocal; round-half-even realized with the 1.5*2^23
    magic-number add). v-domain bounds: Mlo = round(zp')+M, Mhi = Mlo+lv.
  - fake-quant chain per group g (groupwise, tensor_scalar 2x mode):
      ACT: v = Identity(x*(1/s) + M)     (rounds x/s onto the f32 int grid)
      DVE ts1: a = min(max(v, Mlo), Mhi) (clip in v-domain, [128,1] scalars)
      DVE ts2: y = (a - M) * s           (dequant, in-place)
    The v-domain clip is value-equivalent to the reference's
    (clip(r+z,0,q)-z)*s: at the low bound bitwise, at the high bound to
    ~2e-6*s (Mhi rounds q=2^p-1-eps onto the integer grid).
  - sigmoid(upbound/lowbound) is computed host-side with jax (default
    device), matching the reference bitwise.
"""
import sys
import numpy as np

for _p in ("/opt/trn_rl_repo", "/root/.axon_site/_ro/trn_rl_repo"):
    if _p not in sys.path:
        sys.path.append(_p)

from contextlib import ExitStack
import concourse.bass as bass
import concourse.tile as tile
from concourse import bacc, mybir
from concourse.bass_utils import run_bass_kernel_spmd

F32 = mybir.dt.float32
ALU = mybir.AluOpType
ACTF = mybir.ActivationFunctionType

ROWS, COLS, G, NB = 4096, 11008, 128, 86
NCORES = 8
R = ROWS // NCORES    # 512 rows per core
NCH = 2               # col chunks per row-tile
GCH = NB // NCH       # 43 groups per chunk
CH = GCH * G          # 5504 cols per chunk
M = 12582912.0        # 1.5*2^23 round-to-even magic
CLIPMIN, CLIPMAX = 1e-5, 1e4

_PREC = np.array([1] + [2, 3, 4, 3, 2] * 17, dtype=np.int32)

LOOKAHEAD = 3


def _bv(small_ap, width=G):
    """[128, n] AP -> [128, n, width] stride-0 broadcast view."""
    return bass.AP(small_ap.tensor, small_ap.offset,
                   [small_ap.ap[0], small_ap.ap[1], [0, width]])

_LEVELS = None


def _levels_jax():
    """2^p - 1 exactly as the jax reference computes it (default device)."""
    global _LEVELS
    if _LEVELS is None:
        import jax.numpy as jnp
        _LEVELS = np.asarray(
            jnp.exp2(jnp.asarray(_PREC).astype(jnp.float32)) - 1.0
        ).astype(np.float32)
    return _LEVELS


def _build(nc):
    lvj = _levels_jax()
    q1 = float(lvj[0])  # ternary clip-high (~0.99999833)
    x = nc.dram_tensor("x", [R, COLS], F32, kind="ExternalInput").ap()
    su = nc.dram_tensor("su", [R, NB], F32, kind="ExternalInput").ap()
    sl = nc.dram_tensor("sl", [R, NB], F32, kind="ExternalInput").ap()
    su05 = nc.dram_tensor("su05", [R, 1], F32, kind="ExternalInput").ap()
    ilv = nc.dram_tensor("inv_levels", [128, NB], F32, kind="ExternalInput").ap()
    lv = nc.dram_tensor("levels", [128, NB], F32, kind="ExternalInput").ap()
    out = nc.dram_tensor("out", [R, COLS], F32, kind="ExternalOutput").ap()
    NT = R // 128

    I16 = mybir.dt.int16

    with tile.TileContext(nc) as tc, ExitStack() as ctx:
        cpool = ctx.enter_context(tc.tile_pool(name="const", bufs=1))
        xpool = ctx.enter_context(tc.tile_pool(name="xp", bufs=4))
        ypool = ctx.enter_context(tc.tile_pool(name="yp", bufs=2))
        vpool = ctx.enter_context(tc.tile_pool(name="vp", bufs=2))
        rpool = ctx.enter_context(tc.tile_pool(name="rowp", bufs=2))
        spool = ctx.enter_context(tc.tile_pool(name="statp", bufs=6))
        gpool = ctx.enter_context(tc.tile_pool(name="gp", bufs=8))

        lv_t = cpool.tile([128, NB], F32, tag="lv")
        nc.sync.dma_start(lv_t[:], lv[:])
        ilv_t = cpool.tile([128, NB], F32, tag="ilv")
        nc.sync.dma_start(ilv_t[:], ilv[:])
        Mb = cpool.tile([128, 1], F32, tag="Mb")
        nc.vector.memset(Mb[:], M)

        chunks = [(rt, c) for rt in range(NT) for c in range(NCH)]
        n = len(chunks)
        state = {}
        rowstate = {}

        def stage_front(k):
            rt, c = chunks[k]
            if c == 0:
                sut = rpool.tile([128, NB], F32, tag="su")
                nc.sync.dma_start(sut[:], su[rt * 128:(rt + 1) * 128, :])
                slt = rpool.tile([128, NB], F32, tag="sl")
                nc.sync.dma_start(slt[:], sl[rt * 128:(rt + 1) * 128, :])
                s5t = rpool.tile([128, 1], F32, tag="su05")
                nc.sync.dma_start(s5t[:], su05[rt * 128:(rt + 1) * 128, :])
                rowstate[rt] = (sut, slt, s5t)
            sut, slt, s5t = rowstate[rt]
            xt = xpool.tile([128, CH], F32, tag="x")
            rmin = spool.tile([128, GCH], F32, tag="rmin")
            rmax = spool.tile([128, GCH], F32, tag="rmax")
            xsmax = spool.tile([128, GCH], F32, tag="xsmax")
            xsmin = spool.tile([128, GCH], F32, tag="xsmin")
            diff = spool.tile([128, GCH], F32, tag="diff")
            scale_r = spool.tile([128, GCH], F32, tag="scale_r")
            rcp = spool.tile([128, GCH], F32, tag="rcp")
            t1 = spool.tile([128, GCH], F32, tag="t1")
            t2 = spool.tile([128, GCH], F32, tag="t2")
            Mlo = spool.tile([128, GCH], F32, tag="Mlo")
            Mhi = spool.tile([128, GCH], F32, tag="Mhi")
            scl = spool.tile([128, GCH], F32, tag="scl")
            rs = spool.tile([128, GCH], F32, tag="rs")

            # first chunk: fine sub-spans so ACT can start ~7us in, not ~26us
            spans = [(0, 11), (11, 11), (22, 11), (33, 10)] if k == 0 else [(0, GCH)]
            for s0, sw in spans:
                ssl = slice(s0, s0 + sw)
                gsl = slice(c * GCH + s0, c * GCH + s0 + sw)
                csl = slice(s0 * G, (s0 + sw) * G)
                for q in range(4):
                    nc.sync.dma_start(
                        xt[q * 32:(q + 1) * 32, csl],
                        x[rt * 128 + q * 32:rt * 128 + (q + 1) * 32,
                          c * CH + s0 * G:c * CH + (s0 + sw) * G])
                xv = xt[:, csl].rearrange("p (g j) -> p g j", j=G)
                nc.vector.tensor_reduce(rmin[:, ssl], xv, axis=mybir.AxisListType.X, op=ALU.min)
                nc.vector.tensor_reduce(rmax[:, ssl], xv, axis=mybir.AxisListType.X, op=ALU.max)
                nc.vector.tensor_tensor(xsmax[:, ssl], sut[:, gsl], rmax[:, ssl], op=ALU.mult)
                nc.vector.tensor_tensor(xsmin[:, ssl], slt[:, gsl], rmin[:, ssl], op=ALU.mult)
                nc.vector.tensor_tensor(diff[:, ssl], xsmax[:, ssl], xsmin[:, ssl], op=ALU.subtract)
                nc.vector.tensor_tensor(scale_r[:, ssl], diff[:, ssl], ilv_t[:, gsl], op=ALU.mult)
                nc.vector.reciprocal(rcp[:, ssl], scale_r[:, ssl])
                nc.vector.tensor_tensor(t1[:, ssl], xsmin[:, ssl], rcp[:, ssl], op=ALU.mult)
                nc.vector.tensor_scalar(t2[:, ssl], t1[:, ssl], -CLIPMAX, CLIPMAX,
                                        op0=ALU.max, op1=ALU.min)
                # Mlo = round_half_even(t2) + M  (one f32 add does both)
                nc.vector.tensor_scalar(Mlo[:, ssl], t2[:, ssl], M, None, op0=ALU.add)
                # Mhi = Mlo + levels (rounds q onto the integer grid at ulp(M)=1)
                nc.vector.tensor_tensor(Mhi[:, ssl], Mlo[:, ssl], lv_t[:, gsl], op=ALU.add)
                nc.vector.tensor_scalar(scl[:, ssl], scale_r[:, ssl], CLIPMIN, CLIPMAX,
                                        op0=ALU.max, op1=ALU.min)
                nc.vector.reciprocal(rs[:, ssl], scl[:, ssl])

                if c == 0 and s0 == 0:
                    # ternary stats: only need span 0's data; issue them here
                    # so the ACT sign op (head of this chunk's ACT queue)
                    # unblocks as early as possible
                    x0v = xt[:, 0:G].rearrange("p (g j) -> p g j", j=G)
                    rsum = spool.tile([128, 1], F32, tag="rsum")
                    nc.vector.tensor_reduce(rsum[:], x0v, axis=mybir.AxisListType.X,
                                            op=ALU.add)
                    rabs = spool.tile([128, 1], F32, tag="rabs")
                    nc.vector.tensor_reduce(rabs[:], x0v, axis=mybir.AxisListType.X,
                                            op=ALU.add, apply_absolute_value=True)
                    nzt_a = spool.tile([128, 1], F32, tag="nzt_a")
                    nc.vector.tensor_scalar(nzt_a[:], rsum[:], -1.0 / 128.0, -CLIPMAX,
                                            op0=ALU.mult, op1=ALU.max)
                    nzt = spool.tile([128, 1], F32, tag="nzt")
                    nc.vector.tensor_scalar(nzt[:], nzt_a[:], CLIPMAX, None, op0=ALU.min)
                    sta = spool.tile([128, 1], F32, tag="sta")
                    nc.vector.tensor_scalar(sta[:], rabs[:], 1.0 / 128.0, s5t[:],
                                            op0=ALU.mult, op1=ALU.mult)
                    stt = spool.tile([128, 1], F32, tag="stt")
                    nc.vector.tensor_scalar(stt[:], sta[:], CLIPMIN, CLIPMAX,
                                            op0=ALU.max, op1=ALU.min)

            st = {"xt": xt, "rs": rs, "scl": scl, "Mlo": Mlo, "Mhi": Mhi}
            if c == 0:
                st["nzt"] = nzt
                st["stt"] = stt
            state[k] = st

        def back_span(k, ga, gb, last_span):
            """Process groups [ga, gb) of chunk k: ACT per group into the
            fullwidth v tile, then span-wide clip + dequant + output DMA."""
            rt, c = chunks[k]
            st = state[k]
            xt, rs, scl = st["xt"], st["rs"], st["scl"]
            Mlo, Mhi = st["Mlo"], st["Mhi"]
            yt, g0, vt = st["yt"], st["g0"], st["vt"]
            for g in range(max(ga, g0), gb):
                nc.scalar.activation(vt[:, g * G:(g + 1) * G],
                                     xt[:, g * G:(g + 1) * G], ACTF.Identity,
                                     bias=Mb[:], scale=rs[:, g:g + 1])
            dga = max(ga, g0)
            yv = yt[:, dga * G:gb * G].rearrange("p (g j) -> p g j", j=G)
            vv = vt[:, dga * G:gb * G].rearrange("p (g j) -> p g j", j=G)
            nc.vector.tensor_tensor(yv, vv, _bv(Mlo[:, dga:gb]), op=ALU.max)
            nc.vector.tensor_tensor(yv, yv, _bv(Mhi[:, dga:gb]), op=ALU.min)
            nc.vector.scalar_tensor_tensor(yv, yv, M, _bv(scl[:, dga:gb]),
                                           op0=ALU.subtract, op1=ALU.mult)
            nc.sync.dma_start(
                out[rt * 128:(rt + 1) * 128, c * CH + ga * G:c * CH + gb * G],
                yt[:, ga * G:gb * G])
            if last_span:
                state.pop(k)

        def back_open(k):
            """Allocate the y/v tiles and emit the ternary group."""
            rt, c = chunks[k]
            st = state[k]
            yt = ypool.tile([128, CH], F32, tag="y")
            vt = vpool.tile([128, CH], F32, tag="vt")
            st["yt"] = yt
            st["vt"] = vt
            g0 = 0
            if c == 0:
                # ternary group 0: sign((x - z)) clipped, * scale_t
                v0 = gpool.tile([128, G], F32, tag="v0")
                nc.scalar.sign(v0[:], st["xt"][:, 0:G], bias=st["nzt"][:])
                nc.vector.tensor_scalar(yt[:, 0:G], v0[:], q1, st["stt"][:],
                                        op0=ALU.min, op1=ALU.mult)
                g0 = 1
            st["g0"] = g0

        def back_bounds(k):
            return [11, 22, 33, GCH] if k == n - 1 else [GCH // 2, GCH]

        # Interleave: each chunk's front (reduces+stats) issues between the
        # first and remaining back spans of the chunk LOOKAHEAD earlier, so
        # the long DVE reduces fill the ACT-paced clip gaps.
        for k in range(n + LOOKAHEAD):
            kb = k - LOOKAHEAD
            if kb >= 0:
                back_open(kb)
                b = back_bounds(kb)
                back_span(kb, 0, b[0], last_span=False)
            if k < n:
                stage_front(k)
            if kb >= 0:
                b = back_bounds(kb)
                prev = b[0]
                for bb in b[1:]:
                    back_span(kb, prev, bb, last_span=(bb == GCH))
                    prev = bb
    return nc


_COMPILED = None


def _get_compiled():
    global _COMPILED
    if _COMPILED is None:
        nc = bacc.Bacc("TRN2", target_bir_lowering=False, debug=False)
        _build(nc)
        nc.compile()
        _COMPILED = nc
    return _COMPILED


def kernel(x, upbound_factor, lowbound_factor):
    import jax, jax.numpy as jnp
    x = np.ascontiguousarray(np.asarray(x, dtype=np.float32))
    up = np.asarray(upbound_factor, dtype=np.float32)
    low = np.asarray(lowbound_factor, dtype=np.float32)
    assert x.shape == (ROWS, COLS) and up.shape == (ROWS, NB) and low.shape == (ROWS, NB)

    # host precompute (matches the reference's own jax ops bitwise)
    su = np.asarray(jax.nn.sigmoid(jnp.asarray(up))).astype(np.float32)
    sl = np.asarray(jax.nn.sigmoid(jnp.asarray(low))).astype(np.float32)
    su05 = (su[:, 0:1] + np.float32(0.5)).astype(np.float32)
    lvj = _levels_jax()
    lv = np.ascontiguousarray(np.broadcast_to(lvj[None, :], (128, NB)), dtype=np.float32)
    ilv = np.ascontiguousarray(
        np.broadcast_to((np.float32(1.0) / lvj)[None, :], (128, NB)), dtype=np.float32)

    in_maps = []
    for i in range(NCORES):
        r0, r1 = i * R, (i + 1) * R
        in_maps.append({
            "x": np.ascontiguousarray(x[r0:r1]),
            "su": np.ascontiguousarray(su[r0:r1]),
            "sl": np.ascontiguousarray(sl[r0:r1]),
            "su05": np.ascontiguousarray(su05[r0:r1]),
            "inv_levels": ilv,
            "levels": lv,
        })

    nc = _get_compiled()
    res = run_bass_kernel_spmd(nc, in_maps, core_ids=list(range(NCORES)), trace=False)
    return np.concatenate([np.asarray(res.results[i]["out"], dtype=np.float32)
                           for i in range(NCORES)], axis=0)
